# revision 14
# baseline (speedup 1.0000x reference)
"""Trainium2 Bass kernel for causal multi-head attention with RoPE.

Problem: B=4, S=2048, D=1024, H=16, hd=64.
  q,k,v = x @ W_{q,k,v}; q,k roped; causal attention with softmax(scores/hd);
  out = ctx @ W_o. Returns (out, roped_k, v).

Sharding: 8 cores = 4 batches x 2 head-halves. Core c handles batch c//2 and
heads [ (c%2)*8, (c%2)*8+8 ). Each core computes its 8 heads' k/v (outputs)
and a partial out (its heads' contribution to the out-projection); the host
sums the two partials per batch (the "all-reduce" of the row-sharded W_o).

On-core layout (per core):
  xT [D, S] (host-pre-transposed x[b]) streamed per 512-col chunk.
  qT/kT computed in [d', s] layout (heads on partitions, 2 heads per 128-tile);
  RoPE applied in that layout via partition-shifted copies + cos/sin tables.
  v computed in natural [s, d'] layout (it is both the output layout and the
  ctx-matmul lhsT layout).
  Scores are computed transposed (scoresT[sk, sq]) so softmax normalization is
  done via a col-packed ones-matmul (denominator lands in psum row 64) and the
  causal structure is realized by restricting matmul N-ranges per key block.
  exp on ACT reads scores from PSUM directly (scale=1/hd fused).
  ctx accumulates transposed (ctxT[d', s]) which feeds the out-projection as
  rhs directly; k output needs [s, hd] so roped kT tiles are PE-transposed.
"""

import sys

sys.path.insert(0, "/opt/trn_rl_repo")

import numpy as np

P = 128
S = 2048
D = 1024
HD = 64
H_CORE = 8          # heads per core
NT = 4              # head-pair tiles per core (2 heads of 64 rows each)
DC = 8              # contraction chunks (D / 128)
CH = 512            # sequence chunk
NCH = S // CH       # 4 chunks
SB = S // P         # 16 s-blocks
N_CORES = 8

_CACHE = {}


def _build_nc():
    import concourse.bacc as bacc
    import concourse.mybir as mybir
    from concourse.tile import TileContext
    from concourse.masks import make_identity

    f32 = mybir.dt.float32
    f32r = mybir.dt.float32r
    EXP = mybir.ActivationFunctionType.Exp

    nc = bacc.Bacc("TRN2", target_bir_lowering=False)

    # ---- DRAM I/O ----
    xT = nc.dram_tensor("xT", [D, S], f32r, kind="ExternalInput")
    wq = nc.dram_tensor("wq", [D, 512], f32r, kind="ExternalInput")
    wk = nc.dram_tensor("wk", [D, 512], f32r, kind="ExternalInput")
    wv = nc.dram_tensor("wv", [D, 512], f32r, kind="ExternalInput")
    wo = nc.dram_tensor("wo", [512, D], f32r, kind="ExternalInput")
    cos2 = nc.dram_tensor("cos2", [P, S], f32, kind="ExternalInput")
    sinn = nc.dram_tensor("sinn", [P, S], f32, kind="ExternalInput")
    tri = nc.dram_tensor("tri", [P, P], f32, kind="ExternalInput")
    k_out = nc.dram_tensor("k_out", [S, H_CORE * HD], f32, kind="ExternalOutput")
    v_out = nc.dram_tensor("v_out", [S, H_CORE * HD], f32, kind="ExternalOutput")
    out_p = nc.dram_tensor("out_p", [S, D], f32, kind="ExternalOutput")

    xT3 = xT.rearrange("(dc p) s -> p dc s", p=P)        # [128, 8, 2048]
    wq3 = wq.rearrange("(dc p) m -> p dc m", p=P)        # [128, 8, 512]
    wk3 = wk.rearrange("(dc p) m -> p dc m", p=P)
    wv3 = wv.rearrange("(dc p) m -> p dc m", p=P)
    wo3 = wo.rearrange("(t p) n -> p t n", p=P)          # [128, 4, 1024]

    with TileContext(nc) as tc:
        with tc.tile_pool(name="wts", bufs=1) as wts, \
             tc.tile_pool(name="tab", bufs=1) as tab, \
             tc.tile_pool(name="kv", bufs=1) as kv, \
             tc.tile_pool(name="xp", bufs=1) as xp, \
             tc.tile_pool(name="qp", bufs=1) as qp, \
             tc.tile_pool(name="cp", bufs=1) as cp, \
             tc.tile_pool(name="ex", bufs=4) as expool, \
             tc.tile_pool(name="tmp", bufs=1) as tmp, \
             tc.tile_pool(name="sm", bufs=2) as sm, \
             tc.tile_pool(name="ps", bufs=5, space="PSUM") as psp, \
             tc.tile_pool(name="psc", bufs=2, space="PSUM") as psc, \
             tc.tile_pool(name="pst", bufs=1, space="PSUM") as pst:

            # ---- constants / weights (loaded once) ----
            def load_w(pool, shape, src_ap, name):
                t = pool.tile(shape, f32r, tag=name, name=name)
                nc.sync.dma_start(t[:], src_ap)
                return t

            wq_sb = load_w(wts, [P, DC, 512], wq3, "wq_sb")
            xc0 = xp.tile([P, DC, CH], f32r, tag="xc", name="xc")
            nc.sync.dma_start(xc0[:], xT3[:, :, 0:CH])
            wk_sb = load_w(wts, [P, DC, 512], wk3, "wk_sb")
            wv_sb = load_w(wts, [P, DC, 512], wv3, "wv_sb")
            wo_sb = load_w(wts, [P, NT, 1024], wo3, "wo_sb")

            tri_sb = tab.tile([P, P], f32)
            nc.sync.dma_start(tri_sb[:], tri[:])
            ident_f = tab.tile([P, P], f32)
            make_identity(nc, ident_f[:])
            ident = tab.tile([P, P], f32r)
            nc.vector.tensor_copy(out=ident[:], in_=ident_f[:])
            onesrow_f = tab.tile([1, HD], f32)
            nc.vector.memset(onesrow_f[:], 1.0)
            onesrow = tab.tile([1, HD], f32r)
            nc.vector.tensor_copy(out=onesrow[:], in_=onesrow_f[:])

            # persistent K/V cache for this core
            kTr = kv.tile([P, NT, S], f32r)              # roped kT, 2 heads per tile row-half
            # natural v, free = 8 heads x (64 v-cols + 1 ones-col for the
            # softmax denominator row of the ctx matmul)
            v_sb = kv.tile([P, SB, 8 * 65], f32r)
            ones_f = tab.tile([P, 1], f32)
            nc.vector.memset(ones_f[:], 1.0)
            for g in range(H_CORE):
                nc.vector.tensor_copy(
                    out=v_sb[:, :, g * 65 + 64:g * 65 + 65],
                    in_=ones_f[:, None, :].to_broadcast((P, SB, 1)),
                )

            def rope_inplace(ap, ccols):
                # ap: [128, 512] slice, heads stacked 2x64 on partitions
                cs = sm.tile([P, CH], f32, tag="costile", name="costile")
                nc.sync.dma_start(cs[:], cos2[:, ccols])
                sn = sm.tile([P, CH], f32, tag="sintile", name="sintile")
                nc.sync.dma_start(sn[:], sinn[:, ccols])
                rot = tmp.tile([P, CH], f32, tag="rot", name="rot", bufs=2)
                nc.vector.tensor_copy(out=rot[0:32], in_=ap[32:64])
                nc.vector.tensor_copy(out=rot[32:64], in_=ap[0:32])
                nc.vector.tensor_copy(out=rot[64:96], in_=ap[96:128])
                nc.vector.tensor_copy(out=rot[96:128], in_=ap[64:96])
                nc.vector.tensor_mul(out=rot[:], in0=rot[:], in1=sn[:])
                nc.vector.tensor_mul(out=ap, in0=ap, in1=cs[:])
                nc.vector.tensor_add(out=ap, in0=ap, in1=rot[:])

            for c in range(NCH):
                ccols = slice(c * CH, (c + 1) * CH)
                jmax = 4 * c + 3

                # ---- load x chunk (chunk 0 prefetched before weight loads) ----
                if c == 0:
                    xc = xc0
                else:
                    xc = xp.tile([P, DC, CH], f32r, tag="xc", name="xc")
                    nc.sync.dma_start(xc[:], xT3[:, :, ccols])

                qTc = qp.tile([P, NT, CH], f32r, tag="qTc", name="qTc")
                ctxTc = cp.tile([P, NT, CH], f32r, tag="ctxTc", name="ctxTc")

                # ---- q/k projections for this chunk ----
                for t in range(NT):
                    ps_q = psp.tile([P, CH], f32, tag="ps", name="ps_q")
                    for dc in range(DC):
                        nc.tensor.matmul(
                            ps_q[:], wq_sb[:, dc, t * P:(t + 1) * P], xc[:, dc, :],
                            start=(dc == 0), stop=(dc == DC - 1),
                        )
                    nc.scalar.copy(qTc[:, t, :], ps_q[:])
                    ps_k = psp.tile([P, CH], f32, tag="ps", name="ps_k")
                    for dc in range(DC):
                        nc.tensor.matmul(
                            ps_k[:], wk_sb[:, dc, t * P:(t + 1) * P], xc[:, dc, :],
                            start=(dc == 0), stop=(dc == DC - 1),
                        )
                    nc.scalar.copy(kTr[:, t, ccols], ps_k[:])

                # ---- rope q and k chunks ----
                for t in range(NT):
                    rope_inplace(qTc[:, t, :], ccols)
                    rope_inplace(kTr[:, t, ccols], ccols)

                # ---- v projection for this chunk ----
                for sbl in range(4):
                    sb_g = c * 4 + sbl
                    ps_v = psp.tile([P, CH], f32, tag="ps", name="ps_v")
                    for dc in range(DC):
                        nc.tensor.matmul(
                            ps_v[:], xc[:, dc, sbl * P:(sbl + 1) * P], wv_sb[:, dc, :],
                            start=(dc == 0), stop=(dc == DC - 1),
                        )
                    v_dst = v_sb[:, sb_g, :].rearrange("p (h c) -> p h c", c=65)
                    nc.scalar.copy(
                        v_dst[:, :, 0:HD],
                        ps_v.rearrange("p (h c) -> p h c", c=HD),
                    )
                    nc.sync.dma_start(
                        v_out[sb_g * P:(sb_g + 1) * P, :],
                        v_dst[:, :, 0:HD].bitcast(f32),
                    )

                # ---- k output: PE-transpose roped kT chunk ----
                for t in range(NT):
                    for sbl in range(4):
                        sb_g = c * 4 + sbl
                        ps_t = pst.tile([P, P], f32r, tag="pst", name="ps_t")
                        nc.tensor.transpose(
                            ps_t[:], kTr[:, t, sb_g * P:(sb_g + 1) * P], ident[:]
                        )
                        ko = sm.tile([P, P], f32, tag="ko", name="ko")
                        nc.vector.tensor_copy(out=ko[:], in_=ps_t[:])
                        nc.sync.dma_start(
                            k_out[sb_g * P:(sb_g + 1) * P, 2 * t * HD:(2 * t + 2) * HD],
                            ko[:],
                        )

                # ---- attention for sq chunk c ----
                for t in range(NT):
                    ps_ctx = [
                        psc.tile([P, CH], f32, tag="psc", name="ps_ctx")
                        for _ in range(2)
                    ]

                    def emit_scores(j, t=t, c=c):
                        start_col = max(0, P * j - CH * c)
                        exs = []
                        for hl in range(2):
                            hr = slice(64 * hl, 64 * hl + 64)
                            ps_s = psp.tile([P, CH], f32, tag="ps", name="ps_s")
                            nc.tensor.matmul(
                                ps_s[:, start_col:],
                                kTr[hr, t, j * P:(j + 1) * P],
                                qTc[hr, t, start_col:],
                                start=True, stop=True,
                            )
                            ex = expool.tile([P, CH], f32r, tag="ex", name="ex")
                            nc.scalar.activation(
                                ex[:, start_col:], ps_s[:, start_col:], EXP,
                                scale=1.0 / HD,
                            )
                            if j >= 4 * c:
                                # diagonal block: zero the strict lower triangle
                                nc.vector.tensor_mul(
                                    out=ex[:, start_col:start_col + P],
                                    in0=ex[:, start_col:start_col + P],
                                    in1=tri_sb[:],
                                )
                            exs.append(ex)
                        return exs

                    def emit_ctx(j, exs, t=t, c=c, jmax=jmax, ps_ctx=ps_ctx):
                        start_col = max(0, P * j - CH * c)
                        for hl in range(2):
                            g = 2 * t + hl
                            nc.tensor.matmul(
                                ps_ctx[hl][0:65, start_col:],
                                v_sb[:, j, g * 65:(g + 1) * 65],
                                exs[hl][:, start_col:],
                                start=(j == 0), stop=(j == jmax),
                            )

                    pending = emit_scores(0)
                    for j in range(jmax + 1):
                        nxt = emit_scores(j + 1) if j < jmax else None
                        emit_ctx(j, pending)
                        pending = nxt
                    # normalize: ctxT = ctx_unnorm * (1/denom) broadcast
                    for hl in range(2):
                        rr = sm.tile([1, CH], f32, tag="rr", name="rr")
                        nc.vector.reciprocal(rr[:], ps_ctx[hl][64:65, :])
                        rrr = sm.tile([1, CH], f32r, tag="rrr", name="rrr")
                        nc.vector.tensor_copy(out=rrr[:], in_=rr[:])
                        ps_b = psp.tile([P, CH], f32, tag="ps", name="ps_b")
                        nc.tensor.matmul(
                            ps_b[0:64, :], onesrow[:], rrr[:], start=True, stop=True
                        )
                        rb = sm.tile([64, CH], f32, tag="rb", name="rb")
                        nc.scalar.copy(rb[:], ps_b[0:64, :])
                        nc.vector.tensor_mul(
                            out=ctxTc[64 * hl:64 * hl + 64, t, :],
                            in0=ps_ctx[hl][0:64, :],
                            in1=rb[:],
                        )

                # ---- out projection for this chunk ----
                for sbl in range(4):
                    for no in range(2):
                        ps_o = psp.tile([P, CH], f32, tag="ps", name="ps_o")
                        for t in range(NT):
                            nc.tensor.matmul(
                                ps_o[:],
                                ctxTc[:, t, sbl * P:(sbl + 1) * P],
                                wo_sb[:, t, no * CH:(no + 1) * CH],
                                start=(t == 0), stop=(t == NT - 1),
                            )
                        ob = sm.tile([P, CH], f32, tag="ob", name="ob")
                        nc.scalar.copy(ob[:], ps_o[:])
                        nc.sync.dma_start(
                            out_p[(c * 4 + sbl) * P:(c * 4 + sbl + 1) * P,
                                  no * CH:(no + 1) * CH],
                            ob[:],
                        )

    nc.compile()
    return nc


def _host_tables():
    inv_freq = 1.0 / (10000.0 ** (np.arange(0, HD, 2, dtype=np.float64) / HD))  # [32]
    pos = np.arange(S, dtype=np.float64)
    ang = inv_freq[:, None] * pos[None, :]          # [32, S]
    ang = np.concatenate([ang, ang], axis=0)        # [64, S]
    cos = np.cos(ang)
    sin = np.sin(ang)
    cos2 = np.concatenate([cos, cos], axis=0).astype(np.float32)   # [128, S]
    sin_signed = sin.copy()
    sin_signed[0:32] *= -1.0                        # rot rows 0:32 carry -x2
    sinn = np.concatenate([sin_signed, sin_signed], axis=0).astype(np.float32)
    tri = (np.arange(P)[:, None] <= np.arange(P)[None, :]).astype(np.float32)
    return cos2, sinn, tri


def kernel(x, W_q, W_k, W_v, W_o, mask, n_heads):
    from concourse.bass_utils import run_bass_kernel_spmd

    x = np.asarray(x, dtype=np.float32)
    W_q = np.asarray(W_q, dtype=np.float32)
    W_k = np.asarray(W_k, dtype=np.float32)
    W_v = np.asarray(W_v, dtype=np.float32)
    W_o = np.asarray(W_o, dtype=np.float32)
    B = x.shape[0]
    H = 16

    if "nc" not in _CACHE:
        _CACHE["nc"] = _build_nc()
    nc = _CACHE["nc"]

    cos2, sinn, tri = _host_tables()

    in_maps = []
    for cid in range(N_CORES):
        b = cid // 2
        hh = cid % 2
        cols = slice(hh * 512, (hh + 1) * 512)
        in_maps.append({
            "xT": np.ascontiguousarray(x[b].T),
            "wq": np.ascontiguousarray(W_q[:, cols]),
            "wk": np.ascontiguousarray(W_k[:, cols]),
            "wv": np.ascontiguousarray(W_v[:, cols]),
            "wo": np.ascontiguousarray(W_o[cols, :]),
            "cos2": cos2,
            "sinn": sinn,
            "tri": tri,
        })

    import time as _time

    res = None
    for attempt in range(5):
        try:
            res = run_bass_kernel_spmd(nc, in_maps, core_ids=list(range(N_CORES)))
            break
        except Exception:
            if attempt == 4:
                raise
            _time.sleep(90)
    assert res is not None

    out = np.zeros((B, S, D), dtype=np.float32)
    k_full = np.zeros((B, H, S, HD), dtype=np.float32)
    v_full = np.zeros((B, H, S, HD), dtype=np.float32)
    for cid in range(N_CORES):
        b = cid // 2
        hh = cid % 2
        r = res.results[cid]
        out[b] += r["out_p"]
        k_full[b, hh * 8:(hh + 1) * 8] = (
            r["k_out"].reshape(S, 8, HD).transpose(1, 0, 2)
        )
        v_full[b, hh * 8:(hh + 1) * 8] = (
            r["v_out"].reshape(S, 8, HD).transpose(1, 0, 2)
        )

    return out, k_full, v_full


# revision 25
# speedup vs baseline: 1.2711x; 1.2711x over previous
"""Trainium2 Bass kernel for causal multi-head attention with RoPE.

Problem: B=4, S=2048, D=1024, H=16, hd=64.
  q,k,v = x @ W_{q,k,v}; q,k roped; causal attention with softmax(scores/hd);
  out = ctx @ W_o. Returns (out, roped_k, v).

Sharding: 8 cores = 4 batches x 2 head-halves. Core c handles batch c//2 and
heads [ (c%2)*8, (c%2)*8+8 ). Each core computes its 8 heads' k/v (outputs)
and a partial out (its heads' contribution to the out-projection); the host
sums the two partials per batch (the "all-reduce" of the row-sharded W_o).

On-core layout (per core):
  xT [D, S] (host-pre-transposed x[b]) streamed per 512-col chunk.
  qT/kT computed in [d', s] layout (heads on partitions, 2 heads per 128-tile);
  RoPE applied in that layout via partition-shifted copies + cos/sin tables.
  v computed in natural [s, d'] layout (it is both the output layout and the
  ctx-matmul lhsT layout).
  Scores are computed transposed (scoresT[sk, sq]) so softmax normalization is
  done via a col-packed ones-matmul (denominator lands in psum row 64) and the
  causal structure is realized by restricting matmul N-ranges per key block.
  exp on ACT reads scores from PSUM directly (scale=1/hd fused).
  ctx accumulates transposed (ctxT[d', s]) which feeds the out-projection as
  rhs directly; k output needs [s, hd] so roped kT tiles are PE-transposed.
"""

import sys

sys.path.insert(0, "/opt/trn_rl_repo")

import numpy as np

P = 128
S = 2048
D = 1024
HD = 64
H_CORE = 8          # heads per core
NT = 4              # head-pair tiles per core (2 heads of 64 rows each)
DC = 8              # contraction chunks (D / 128)
CH = 512            # sequence chunk
NCH = S // CH       # 4 chunks
SB = S // P         # 16 s-blocks
N_CORES = 8

_CACHE = {}


def _build_nc():
    import concourse.bacc as bacc
    import concourse.mybir as mybir
    from concourse.tile import TileContext
    from concourse.masks import make_identity

    f32 = mybir.dt.float32
    f32r = mybir.dt.float32r
    EXP = mybir.ActivationFunctionType.Exp

    nc = bacc.Bacc("TRN2", target_bir_lowering=False)

    # ---- DRAM I/O ----
    xT = nc.dram_tensor("xT", [D, S], f32r, kind="ExternalInput")
    wq = nc.dram_tensor("wq", [D, 512], f32r, kind="ExternalInput")
    wk = nc.dram_tensor("wk", [D, 512], f32r, kind="ExternalInput")
    wv = nc.dram_tensor("wv", [D, 512], f32r, kind="ExternalInput")
    wo = nc.dram_tensor("wo", [512, D], f32r, kind="ExternalInput")
    cos2 = nc.dram_tensor("cos2", [P, S], f32, kind="ExternalInput")
    sinn = nc.dram_tensor("sinn", [P, S], f32, kind="ExternalInput")
    tri = nc.dram_tensor("tri", [P, P], f32, kind="ExternalInput")
    k_out = nc.dram_tensor("k_out", [S, H_CORE * HD], f32, kind="ExternalOutput")
    v_out = nc.dram_tensor("v_out", [S, H_CORE * HD], f32, kind="ExternalOutput")
    out_p = nc.dram_tensor("out_p", [S, D], f32, kind="ExternalOutput")

    xT3 = xT.rearrange("(dc p) s -> p dc s", p=P)        # [128, 8, 2048]
    wq3 = wq.rearrange("(dc p) m -> p dc m", p=P)        # [128, 8, 512]
    wk3 = wk.rearrange("(dc p) m -> p dc m", p=P)
    wv3 = wv.rearrange("(dc p) m -> p dc m", p=P)
    wo3 = wo.rearrange("(t p) n -> p t n", p=P)          # [128, 4, 1024]

    with TileContext(nc) as tc:
        with tc.tile_pool(name="wts", bufs=1) as wts, \
             tc.tile_pool(name="tab", bufs=1) as tab, \
             tc.tile_pool(name="kv", bufs=1) as kv, \
             tc.tile_pool(name="xp", bufs=1) as xp, \
             tc.tile_pool(name="qp", bufs=1) as qp, \
             tc.tile_pool(name="cp", bufs=1) as cp, \
             tc.tile_pool(name="ex", bufs=4) as expool, \
             tc.tile_pool(name="tmp", bufs=1) as tmp, \
             tc.tile_pool(name="sm", bufs=2) as sm, \
             tc.tile_pool(name="ps", bufs=5, space="PSUM") as psp, \
             tc.tile_pool(name="psc", bufs=2, space="PSUM") as psc, \
             tc.tile_pool(name="pst", bufs=1, space="PSUM") as pst:

            # ---- constants / weights (loaded once) ----
            def load_w(pool, shape, src_ap, name):
                t = pool.tile(shape, f32r, tag=name, name=name)
                nc.sync.dma_start(t[:], src_ap)
                return t

            wq_sb = load_w(wts, [P, DC, 512], wq3, "wq_sb")
            xc0 = xp.tile([P, DC, CH], f32r, tag="xc", name="xc")
            nc.sync.dma_start(xc0[:], xT3[:, :, 0:CH])
            wk_sb = load_w(wts, [P, DC, 512], wk3, "wk_sb")
            wv_sb = load_w(wts, [P, DC, 512], wv3, "wv_sb")
            wo_sb = load_w(wts, [P, NT, 1024], wo3, "wo_sb")

            tri_sb = tab.tile([P, P], f32)
            nc.sync.dma_start(tri_sb[:], tri[:])
            ident_f = tab.tile([P, P], f32)
            make_identity(nc, ident_f[:])
            ident = tab.tile([P, P], f32r)
            nc.vector.tensor_copy(out=ident[:], in_=ident_f[:])
            onesrow_f = tab.tile([1, HD], f32)
            nc.vector.memset(onesrow_f[:], 1.0)
            onesrow = tab.tile([1, HD], f32r)
            nc.vector.tensor_copy(out=onesrow[:], in_=onesrow_f[:])

            # persistent K/V cache for this core
            kTr = kv.tile([P, NT, S], f32r)              # roped kT, 2 heads per tile row-half
            # natural v, free = 8 heads x (64 v-cols + 1 ones-col for the
            # softmax denominator row of the ctx matmul)
            v_sb = kv.tile([P, SB, 8 * 65], f32r)
            ones_f = tab.tile([P, 1], f32)
            nc.vector.memset(ones_f[:], 1.0)
            for g in range(H_CORE):
                nc.vector.tensor_copy(
                    out=v_sb[:, :, g * 65 + 64:g * 65 + 65],
                    in_=ones_f[:, None, :].to_broadcast((P, SB, 1)),
                )

            def rope_inplace(ap, cs, sn):
                # ap: [128, 512] slice, heads stacked 2x64 on partitions
                rot = tmp.tile([P, CH], f32, tag="rot", name="rot", bufs=2)
                nc.vector.tensor_copy(out=rot[0:32], in_=ap[32:64])
                nc.vector.tensor_copy(out=rot[32:64], in_=ap[0:32])
                nc.vector.tensor_copy(out=rot[64:96], in_=ap[96:128])
                nc.vector.tensor_copy(out=rot[96:128], in_=ap[64:96])
                nc.vector.tensor_mul(out=rot[:], in0=rot[:], in1=sn[:])
                nc.vector.tensor_mul(out=ap, in0=ap, in1=cs[:])
                nc.vector.tensor_add(out=ap, in0=ap, in1=rot[:])

            for c in range(NCH):
                ccols = slice(c * CH, (c + 1) * CH)
                jmax = 4 * c + 3

                # ---- load x chunk (chunk 0 prefetched before weight loads) ----
                if c == 0:
                    xc = xc0
                else:
                    xc = xp.tile([P, DC, CH], f32r, tag="xc", name="xc")
                    nc.sync.dma_start(xc[:], xT3[:, :, ccols])

                qTc = qp.tile([P, NT, CH], f32r, tag="qTc", name="qTc")
                ctxTc = cp.tile([P, NT, CH], f32r, tag="ctxTc", name="ctxTc")

                # ---- q/k projections for this chunk ----
                for t in range(NT):
                    ps_q = psp.tile([P, CH], f32, tag="ps", name="ps_q")
                    for dc in range(DC):
                        nc.tensor.matmul(
                            ps_q[:], wq_sb[:, dc, t * P:(t + 1) * P], xc[:, dc, :],
                            start=(dc == 0), stop=(dc == DC - 1),
                        )
                    nc.scalar.copy(qTc[:, t, :], ps_q[:])
                    ps_k = psp.tile([P, CH], f32, tag="ps", name="ps_k")
                    for dc in range(DC):
                        nc.tensor.matmul(
                            ps_k[:], wk_sb[:, dc, t * P:(t + 1) * P], xc[:, dc, :],
                            start=(dc == 0), stop=(dc == DC - 1),
                        )
                    nc.scalar.copy(kTr[:, t, ccols], ps_k[:])

                # ---- rope q and k chunks ----
                cs = sm.tile([P, CH], f32, tag="costile", name="costile")
                nc.sync.dma_start(cs[:], cos2[:, ccols])
                sn = sm.tile([P, CH], f32, tag="sintile", name="sintile")
                nc.sync.dma_start(sn[:], sinn[:, ccols])
                for t in range(NT):
                    rope_inplace(qTc[:, t, :], cs, sn)
                    rope_inplace(kTr[:, t, ccols], cs, sn)

                # ---- v projection for this chunk ----
                for sbl in range(4):
                    sb_g = c * 4 + sbl
                    ps_v = psp.tile([P, CH], f32, tag="ps", name="ps_v")
                    for dc in range(DC):
                        nc.tensor.matmul(
                            ps_v[:], xc[:, dc, sbl * P:(sbl + 1) * P], wv_sb[:, dc, :],
                            start=(dc == 0), stop=(dc == DC - 1),
                        )
                    v_dst = v_sb[:, sb_g, :].rearrange("p (h c) -> p h c", c=65)
                    nc.scalar.copy(
                        v_dst[:, :, 0:HD],
                        ps_v.rearrange("p (h c) -> p h c", c=HD),
                    )
                    nc.sync.dma_start(
                        v_out[sb_g * P:(sb_g + 1) * P, :],
                        v_dst[:, :, 0:HD].bitcast(f32),
                    )

                # ---- k output: PE-transpose roped kT chunk ----
                for t in range(NT):
                    for sbl in range(4):
                        sb_g = c * 4 + sbl
                        ps_t = pst.tile([P, P], f32r, tag="pst", name="ps_t")
                        nc.tensor.transpose(
                            ps_t[:], kTr[:, t, sb_g * P:(sb_g + 1) * P], ident[:]
                        )
                        ko = sm.tile([P, P], f32, tag="ko", name="ko")
                        nc.vector.tensor_copy(out=ko[:], in_=ps_t[:])
                        nc.sync.dma_start(
                            k_out[sb_g * P:(sb_g + 1) * P, 2 * t * HD:(2 * t + 2) * HD],
                            ko[:],
                        )

                # ---- attention for sq chunk c ----
                for t in range(NT):
                    ps_ctx = [
                        psc.tile([P, CH], f32, tag="psc", name="ps_ctx")
                        for _ in range(2)
                    ]

                    def emit_scores(j, t=t, c=c):
                        start_col = max(0, P * j - CH * c)
                        exs = []
                        for hl in range(2):
                            hr = slice(64 * hl, 64 * hl + 64)
                            ps_s = psp.tile([P, CH], f32, tag="ps", name="ps_s")
                            nc.tensor.matmul(
                                ps_s[:, start_col:],
                                kTr[hr, t, j * P:(j + 1) * P],
                                qTc[hr, t, start_col:],
                                start=True, stop=True,
                            )
                            ex = expool.tile([P, CH], f32r, tag="ex", name="ex")
                            nc.scalar.activation(
                                ex[:, start_col:], ps_s[:, start_col:], EXP,
                                scale=1.0 / HD,
                            )
                            if j >= 4 * c:
                                # diagonal block: zero the strict lower triangle
                                nc.vector.tensor_mul(
                                    out=ex[:, start_col:start_col + P],
                                    in0=ex[:, start_col:start_col + P],
                                    in1=tri_sb[:],
                                )
                            exs.append(ex)
                        return exs

                    def emit_ctx(j, exs, t=t, c=c, jmax=jmax, ps_ctx=ps_ctx):
                        start_col = max(0, P * j - CH * c)
                        for hl in range(2):
                            g = 2 * t + hl
                            nc.tensor.matmul(
                                ps_ctx[hl][0:65, start_col:],
                                v_sb[:, j, g * 65:(g + 1) * 65],
                                exs[hl][:, start_col:],
                                start=(j == 0), stop=(j == jmax),
                            )

                    pending = emit_scores(0)
                    for j in range(jmax + 1):
                        nxt = emit_scores(j + 1) if j < jmax else None
                        emit_ctx(j, pending)
                        pending = nxt
                    # normalize: ctxT = ctx_unnorm * (1/denom) broadcast
                    for hl in range(2):
                        rr = sm.tile([1, CH], f32, tag="rr", name="rr")
                        nc.vector.reciprocal(rr[:], ps_ctx[hl][64:65, :])
                        rrr = sm.tile([1, CH], f32r, tag="rrr", name="rrr")
                        nc.vector.tensor_copy(out=rrr[:], in_=rr[:])
                        ps_b = psp.tile([P, CH], f32, tag="ps", name="ps_b")
                        nc.tensor.matmul(
                            ps_b[0:64, :], onesrow[:], rrr[:], start=True, stop=True
                        )
                        rb = sm.tile([64, CH], f32, tag="rb", name="rb")
                        nc.scalar.copy(rb[:], ps_b[0:64, :])
                        nc.vector.tensor_mul(
                            out=ctxTc[64 * hl:64 * hl + 64, t, :],
                            in0=ps_ctx[hl][0:64, :],
                            in1=rb[:],
                        )

                # ---- out projection for this chunk ----
                for sbl in range(4):
                    for no in range(2):
                        ps_o = psp.tile([P, CH], f32, tag="ps", name="ps_o")
                        for t in range(NT):
                            nc.tensor.matmul(
                                ps_o[:],
                                ctxTc[:, t, sbl * P:(sbl + 1) * P],
                                wo_sb[:, t, no * CH:(no + 1) * CH],
                                start=(t == 0), stop=(t == NT - 1),
                            )
                        ob = sm.tile([P, CH], f32, tag="ob", name="ob")
                        nc.scalar.copy(ob[:], ps_o[:])
                        nc.sync.dma_start(
                            out_p[(c * 4 + sbl) * P:(c * 4 + sbl + 1) * P,
                                  no * CH:(no + 1) * CH],
                            ob[:],
                        )

    nc.compile()
    return nc


def _host_tables():
    inv_freq = 1.0 / (10000.0 ** (np.arange(0, HD, 2, dtype=np.float64) / HD))  # [32]
    pos = np.arange(S, dtype=np.float64)
    ang = inv_freq[:, None] * pos[None, :]          # [32, S]
    ang = np.concatenate([ang, ang], axis=0)        # [64, S]
    cos = np.cos(ang)
    sin = np.sin(ang)
    cos2 = np.concatenate([cos, cos], axis=0).astype(np.float32)   # [128, S]
    sin_signed = sin.copy()
    sin_signed[0:32] *= -1.0                        # rot rows 0:32 carry -x2
    sinn = np.concatenate([sin_signed, sin_signed], axis=0).astype(np.float32)
    tri = (np.arange(P)[:, None] <= np.arange(P)[None, :]).astype(np.float32)
    return cos2, sinn, tri


def kernel(x, W_q, W_k, W_v, W_o, mask, n_heads):
    from concourse.bass_utils import run_bass_kernel_spmd

    x = np.asarray(x, dtype=np.float32)
    W_q = np.asarray(W_q, dtype=np.float32)
    W_k = np.asarray(W_k, dtype=np.float32)
    W_v = np.asarray(W_v, dtype=np.float32)
    W_o = np.asarray(W_o, dtype=np.float32)
    B = x.shape[0]
    H = 16

    if "nc" not in _CACHE:
        _CACHE["nc"] = _build_nc()
    nc = _CACHE["nc"]

    cos2, sinn, tri = _host_tables()

    in_maps = []
    for cid in range(N_CORES):
        b = cid // 2
        hh = cid % 2
        cols = slice(hh * 512, (hh + 1) * 512)
        in_maps.append({
            "xT": np.ascontiguousarray(x[b].T),
            "wq": np.ascontiguousarray(W_q[:, cols]),
            "wk": np.ascontiguousarray(W_k[:, cols]),
            "wv": np.ascontiguousarray(W_v[:, cols]),
            "wo": np.ascontiguousarray(W_o[cols, :]),
            "cos2": cos2,
            "sinn": sinn,
            "tri": tri,
        })

    import time as _time

    res = None
    for attempt in range(5):
        try:
            res = run_bass_kernel_spmd(nc, in_maps, core_ids=list(range(N_CORES)))
            break
        except Exception:
            if attempt == 4:
                raise
            _time.sleep(90)
    assert res is not None

    out = np.zeros((B, S, D), dtype=np.float32)
    k_full = np.zeros((B, H, S, HD), dtype=np.float32)
    v_full = np.zeros((B, H, S, HD), dtype=np.float32)
    for cid in range(N_CORES):
        b = cid // 2
        hh = cid % 2
        r = res.results[cid]
        out[b] += r["out_p"]
        k_full[b, hh * 8:(hh + 1) * 8] = (
            r["k_out"].reshape(S, 8, HD).transpose(1, 0, 2)
        )
        v_full[b, hh * 8:(hh + 1) * 8] = (
            r["v_out"].reshape(S, 8, HD).transpose(1, 0, 2)
        )

    return out, k_full, v_full
